# revision 17
# baseline (speedup 1.0000x reference)
"""CTRNN cell (6 Euler unfolds) on 8 Trainium2 NeuronCores.

Math (per unfold, 6x):
    f     = tanh([x, s] @ W + b)
    s_new = s + 0.1 * (-s + f)  = 0.9*s + 0.1*f

Strategy:
  - Data-parallel over batch: B=8192 -> 1024 rows/core, no cross-core comms.
  - Everything kept TRANSPOSED on-chip (feature dim on SBUF partitions,
    batch on the free dim); host does the cheap numpy transposes.
  - pre = x @ W_top is computed once. Per-unfold matmuls run in *delta*
    form: one PSUM accumulator per output m-tile holds pre + s_k @ W_bot
    across all unfolds, updated with psum += (f_k - s_k) @ (0.1*W_bot).
    That is the 7-logical-matmul FLOP floor.
  - Init matmuls in float32r (fp32 precision, bf16 rate); delta matmuls in
    bf16 (small corrections -> bf16 error is scaled by ~0.1) which also
    enables fast weight loads.
  - PSUM per m-tile is one (128,1024) span (2 banks); matmuls write
    512-wide halves, tanh/DVE read the full row to amortize op overhead.
  - bias is folded into the tanh activation's per-partition bias operand.
  - Input DMAs split across both HWDGE rings (sync + scalar engines);
    f32r rounding casts run on the otherwise-idle scalar engine.
"""

import numpy as np

UNFOLDS = 6
DT = 0.1
B, D, N = 8192, 512, 512
NCORES = 8
BC = B // NCORES          # batch rows per core
CHUNK = 512               # matmul moving-operand free dim (PSUM bank)
NCHUNKS = BC // CHUNK     # 2
P = 128
KT_X = D // P             # k-tiles of W_top
KT_S = N // P             # k-tiles of W_bot
MT = N // P               # m-tiles of the output dim

_compiled_nc = None


def _build_nc():
    import concourse.bass as bass  # noqa: F401
    import concourse.bacc as bacc
    import concourse.tile as tile
    from concourse import mybir

    f32 = mybir.dt.float32
    f32r = mybir.dt.float32r
    bf16 = mybir.dt.bfloat16
    MULT = mybir.AluOpType.mult
    ADD = mybir.AluOpType.add
    TANH = mybir.ActivationFunctionType.Tanh

    nc = bacc.Bacc("TRN2", target_bir_lowering=False, debug=False)

    xT = nc.dram_tensor("xT", [D, BC], f32r, kind="ExternalInput").ap()
    sT = nc.dram_tensor("sT", [N, BC], f32r, kind="ExternalInput").ap()
    Wp_d = nc.dram_tensor("Wp", [P, (D + N) * N // P], f32r,
                          kind="ExternalInput").ap()
    bias = nc.dram_tensor("bias", [N], f32, kind="ExternalInput").ap()
    outT = nc.dram_tensor("outT", [N, BC], f32, kind="ExternalOutput").ap()

    with tile.TileContext(nc) as tc:
        with (
            tc.tile_pool(name="weights", bufs=1) as wpool,
            tc.tile_pool(name="dmain", bufs=3) as dmain,
            tc.tile_pool(name="data", bufs=1) as data,
            tc.tile_pool(name="tmp", bufs=2) as tmpp,
            tc.tile_pool(name="fpool", bufs=3) as fpool,
            tc.tile_pool(name="psum", bufs=1, space="PSUM") as psump,
        ):
            # ---- input DMAs (all f32r-direct, no rounding casts) -----------
            # walrus accepts DMACopy with f32r output as the rounding
            # producer for f32r matmuls (verified on HW: identical result to
            # an explicit DVE cast). W arrives host-packed as (128, 4096) so
            # every DMA has 4KB-contiguous per-partition runs (the DMA queues
            # are descriptor-rate-bound: 2KB rows halve the bandwidth).
            # Load is balanced across SWDGE (~200 GB/s) and the two HWDGE
            # rings; everything lands by ~HBM-bound time.
            NPAIR = (KT_X + KT_S) // 2
            wp = []
            for q in range(NPAIR):
                wd = wpool.tile([P, 2 * N], f32r, tag=f"wp{q}", name=f"wp{q}")
                eng = nc.gpsimd if q < 2 else nc.scalar
                eng.dma_start(wd[:], Wp_d[:, q * 2 * N:(q + 1) * 2 * N])
                wp.append(wd)
            x_sb = []
            for j in range(KT_X):
                t = data.tile([P, BC], f32r, tag=f"x{j}", name=f"x{j}")
                nc.sync.dma_start(t[:], xT[j * P:(j + 1) * P, :])
                x_sb.append(t)
            s_sb = []
            for j in range(KT_S):
                t = data.tile([P, BC], f32r, tag=f"s{j}", name=f"s{j}")
                eng = nc.scalar if j == 3 else nc.gpsimd
                eng.dma_start(t[:], sT[j * P:(j + 1) * P, :])
                s_sb.append(t)
            bias_sb = wpool.tile([P, MT], f32, tag="bias", name="bias_sb")
            nc.gpsimd.dma_start(bias_sb[:], bias.rearrange("(m p) -> p m", p=P))

            # the only casts left: 0.1*W_bot in bf16 for the delta matmuls
            wbp01h = []
            for q in range(NPAIR // 2):
                w = wpool.tile([P, 2 * N], bf16, tag=f"wbph{q}",
                               name=f"wbp01h_{q}")
                nc.scalar.mul(w[:], wp[NPAIR // 2 + q][:], DT)
                wbp01h.append(w)

            def pair_slices(pairs):
                out = []
                for w in pairs:
                    out.append(w[:, 0:N])
                    out.append(w[:, N:2 * N])
                return out

            wt = pair_slices(wp[:NPAIR // 2])       # W_top f32r k-slices
            wbot = pair_slices(wp[NPAIR // 2:])     # W_bot f32r k-slices
            wb01h = pair_slices(wbp01h)             # 0.1*W_bot bf16 k-slices

            # ---- persistent PSUM accumulators: pre + s_k @ W_bot ----------
            # one (128, 1024) span per m-tile = 2 banks; matmuls address
            # 512-wide halves, ACT reads the whole span.
            ps = [psump.tile([P, BC], f32, tag=f"ps{m}", name=f"ps{m}")
                  for m in range(MT)]

            def mm_round(weights, rhs_tiles, start, stop, m_outer=False):
                nkt = len(rhs_tiles)
                order = (
                    [(j, m) for m in range(MT) for j in range(nkt)]
                    if m_outer else
                    [(j, m) for j in range(nkt) for m in range(MT)]
                )
                for j, m in order:
                    for c in range(NCHUNKS):
                        nc.tensor.matmul(
                            ps[m][:, c * CHUNK:(c + 1) * CHUNK],
                            lhsT=weights[j][:, m * P:(m + 1) * P],
                            rhs=rhs_tiles[j][:, c * CHUNK:(c + 1) * CHUNK],
                            start=(start and j == 0),
                            stop=(stop and j == nkt - 1),
                            skip_group_check=True,
                        )

            # init: psum = x @ W_top + s0 @ W_bot, j-interleaved so the PE
            # consumes tiles roughly in arrival order
            def mm_ktile(weights_j, rhs_j, start):
                for m in range(MT):
                    for c in range(NCHUNKS):
                        nc.tensor.matmul(
                            ps[m][:, c * CHUNK:(c + 1) * CHUNK],
                            lhsT=weights_j[:, m * P:(m + 1) * P],
                            rhs=rhs_j[:, c * CHUNK:(c + 1) * CHUNK],
                            start=start, stop=False,
                            skip_group_check=True,
                        )

            for j in range(KT_X):
                mm_ktile(wt[j], x_sb[j][:], start=(j == 0))
                mm_ktile(wbot[j], s_sb[j][:], start=False)

            # ---- unfolds ---------------------------------------------------
            for k in range(UNFOLDS):
                last = k == UNFOLDS - 1
                tmp_t = [tmpp.tile([P, BC], bf16, tag=f"tmp{j}",
                                   name=f"tmp{k}_{j}")
                         for j in range(MT)]
                f_t = [fpool.tile([P, BC], f32, tag=f"f{m}", name=f"f{k}_{m}",
                                  bufs=2)
                       for m in range(MT)]
                for m in range(MT):
                    # f = tanh(psum + bias), full (128,1024) span
                    nc.scalar.activation(
                        f_t[m][:], ps[m][:], TANH,
                        bias=bias_sb[:, m:m + 1], scale=1.0,
                    )
                    # tmp = f - s (bf16 out, feeds the delta matmuls)
                    nc.vector.scalar_tensor_tensor(
                        tmp_t[m][:], s_sb[m][:], -1.0, f_t[m][:],
                        op0=MULT, op1=ADD,
                    )
                    if last:
                        # final state + output DMA per m-tile, ASAP
                        nc.vector.scalar_tensor_tensor(
                            s_sb[m][:], tmp_t[m][:], DT, s_sb[m][:],
                            op0=MULT, op1=ADD,
                        )
                        out_eng = nc.sync if m % 2 == 0 else nc.scalar
                        out_eng.dma_start(outT[m * P:(m + 1) * P, :],
                                          s_sb[m][:].bitcast(f32))
                if not last:
                    # psum += tmp @ (0.1*W_bot)   [bf16]
                    mm_round(wb01h, tmp_t, start=False,
                             stop=(k == UNFOLDS - 2))
                    # s += 0.1 * tmp  (emitted after the matmuls: off the
                    # critical path, fills DVE gaps)
                    for m in range(MT):
                        nc.vector.scalar_tensor_tensor(
                            s_sb[m][:], tmp_t[m][:], DT, s_sb[m][:],
                            op0=MULT, op1=ADD,
                        )

    nc.compile()
    return nc


def _get_nc():
    global _compiled_nc
    if _compiled_nc is None:
        _compiled_nc = _build_nc()
    return _compiled_nc


def make_in_maps(x, s, W, b):
    """Shard + pack host-side: transposed x/s, W packed to (128, 4096) with
    4KB-contiguous per-partition runs (k-tile pairs side by side)."""
    xT = np.ascontiguousarray(x.T)   # (D, B)
    sTf = np.ascontiguousarray(s.T)  # (N, B)
    Wp = np.ascontiguousarray(
        W.reshape(4, 2, P, N).transpose(2, 0, 1, 3).reshape(P, -1))
    in_maps = []
    for c in range(NCORES):
        sl = slice(c * BC, (c + 1) * BC)
        in_maps.append({
            "xT": np.ascontiguousarray(xT[:, sl]),
            "sT": np.ascontiguousarray(sTf[:, sl]),
            "Wp": Wp,
            "bias": b,
        })
    return in_maps


def kernel(**inputs):
    from concourse.bass_utils import run_bass_kernel_spmd

    x = np.asarray(inputs["inputs"], dtype=np.float32)
    s = np.asarray(inputs["state"], dtype=np.float32)
    W = np.ascontiguousarray(np.asarray(inputs["W"], dtype=np.float32))
    b = np.ascontiguousarray(np.asarray(inputs["bias"], dtype=np.float32))

    in_maps = make_in_maps(x, s, W, b)
    nc = _get_nc()
    res = run_bass_kernel_spmd(nc, in_maps, list(range(NCORES))).results
    outT = np.concatenate([res[c]["outT"] for c in range(NCORES)], axis=1)
    out = np.ascontiguousarray(outT.T).astype(np.float32)
    return (out, out)


# revision 18
# speedup vs baseline: 1.0349x; 1.0349x over previous
"""CTRNN cell (6 Euler unfolds) on 8 Trainium2 NeuronCores.

Math (per unfold, 6x):
    f     = tanh([x, s] @ W + b)
    s_new = s + 0.1 * (-s + f)  = 0.9*s + 0.1*f

Strategy:
  - Data-parallel over batch: B=8192 -> 1024 rows/core, no cross-core comms.
  - Everything kept TRANSPOSED on-chip (feature dim on SBUF partitions,
    batch on the free dim); host does the cheap numpy transposes.
  - pre = x @ W_top is computed once. Per-unfold matmuls run in *delta*
    form: one PSUM accumulator per output m-tile holds pre + s_k @ W_bot
    across all unfolds, updated with psum += (f_k - s_k) @ (0.1*W_bot).
    That is the 7-logical-matmul FLOP floor.
  - Init matmuls in float32r (fp32 precision, bf16 rate); delta matmuls in
    bf16 (small corrections -> bf16 error is scaled by ~0.1) which also
    enables fast weight loads.
  - PSUM per m-tile is one (128,1024) span (2 banks); matmuls write
    512-wide halves, tanh/DVE read the full row to amortize op overhead.
  - bias is folded into the tanh activation's per-partition bias operand.
  - Input DMAs split across both HWDGE rings (sync + scalar engines);
    f32r rounding casts run on the otherwise-idle scalar engine.
"""

import numpy as np

UNFOLDS = 6
DT = 0.1
B, D, N = 8192, 512, 512
NCORES = 8
BC = B // NCORES          # batch rows per core
CHUNK = 512               # matmul moving-operand free dim (PSUM bank)
NCHUNKS = BC // CHUNK     # 2
P = 128
KT_X = D // P             # k-tiles of W_top
KT_S = N // P             # k-tiles of W_bot
MT = N // P               # m-tiles of the output dim

_compiled_nc = None


def _build_nc():
    import concourse.bass as bass  # noqa: F401
    import concourse.bacc as bacc
    import concourse.tile as tile
    from concourse import mybir

    f32 = mybir.dt.float32
    f32r = mybir.dt.float32r
    bf16 = mybir.dt.bfloat16
    MULT = mybir.AluOpType.mult
    ADD = mybir.AluOpType.add
    TANH = mybir.ActivationFunctionType.Tanh

    nc = bacc.Bacc("TRN2", target_bir_lowering=False, debug=False)

    xT = nc.dram_tensor("xT", [D, BC], f32r, kind="ExternalInput").ap()
    sT = nc.dram_tensor("sT", [N, BC], f32r, kind="ExternalInput").ap()
    Wp_d = nc.dram_tensor("Wp", [P, (D + N) * N // P], f32r,
                          kind="ExternalInput").ap()
    bias = nc.dram_tensor("bias", [N], f32, kind="ExternalInput").ap()
    outT = nc.dram_tensor("outT", [N, BC], f32, kind="ExternalOutput").ap()

    with tile.TileContext(nc) as tc:
        with (
            tc.tile_pool(name="weights", bufs=1) as wpool,
            tc.tile_pool(name="dmain", bufs=3) as dmain,
            tc.tile_pool(name="data", bufs=1) as data,
            tc.tile_pool(name="tmp", bufs=2) as tmpp,
            tc.tile_pool(name="fpool", bufs=3) as fpool,
            tc.tile_pool(name="psum", bufs=1, space="PSUM") as psump,
        ):
            # ---- input DMAs (all f32r-direct, no rounding casts) -----------
            # walrus accepts DMACopy with f32r output as the rounding
            # producer for f32r matmuls (verified on HW: identical result to
            # an explicit DVE cast). W arrives host-packed as (128, 4096) so
            # every DMA has 4KB-contiguous per-partition runs (the DMA queues
            # are descriptor-rate-bound: 2KB rows halve the bandwidth).
            # Load is balanced across SWDGE (~200 GB/s) and the two HWDGE
            # rings; everything lands by ~HBM-bound time.
            NPAIR = (KT_X + KT_S) // 2
            wp = []
            for q in range(NPAIR):
                wd = wpool.tile([P, 2 * N], f32r, tag=f"wp{q}", name=f"wp{q}")
                eng = nc.gpsimd if q < 2 else nc.scalar
                eng.dma_start(wd[:], Wp_d[:, q * 2 * N:(q + 1) * 2 * N])
                wp.append(wd)
            x_sb = []
            for j in range(KT_X):
                t = data.tile([P, BC], f32r, tag=f"x{j}", name=f"x{j}")
                nc.sync.dma_start(t[:], xT[j * P:(j + 1) * P, :])
                x_sb.append(t)
            s_sb = []
            for j in range(KT_S):
                t = data.tile([P, BC], f32r, tag=f"s{j}", name=f"s{j}")
                eng = nc.scalar if j == 3 else nc.gpsimd
                eng.dma_start(t[:], sT[j * P:(j + 1) * P, :])
                s_sb.append(t)
            bias_sb = wpool.tile([P, MT], f32, tag="bias", name="bias_sb")
            nc.gpsimd.dma_start(bias_sb[:], bias.rearrange("(m p) -> p m", p=P))

            # the only casts left: 0.1*W_bot in bf16 for the delta matmuls
            wbp01h = []
            for q in range(NPAIR // 2):
                w = wpool.tile([P, 2 * N], bf16, tag=f"wbph{q}",
                               name=f"wbp01h_{q}")
                nc.scalar.mul(w[:], wp[NPAIR // 2 + q][:], DT)
                wbp01h.append(w)

            def pair_slices(pairs):
                out = []
                for w in pairs:
                    out.append(w[:, 0:N])
                    out.append(w[:, N:2 * N])
                return out

            wt = pair_slices(wp[:NPAIR // 2])       # W_top f32r k-slices
            wbot = pair_slices(wp[NPAIR // 2:])     # W_bot f32r k-slices
            wb01h = pair_slices(wbp01h)             # 0.1*W_bot bf16 k-slices

            # ---- persistent PSUM accumulators: pre + s_k @ W_bot ----------
            # one (128, 1024) span per m-tile = 2 banks; matmuls address
            # 512-wide halves, ACT reads the whole span.
            ps = [psump.tile([P, BC], f32, tag=f"ps{m}", name=f"ps{m}")
                  for m in range(MT)]

            def mm_round(weights, rhs_tiles, start, stop, m_outer=False):
                nkt = len(rhs_tiles)
                order = (
                    [(j, m) for m in range(MT) for j in range(nkt)]
                    if m_outer else
                    [(j, m) for j in range(nkt) for m in range(MT)]
                )
                for j, m in order:
                    for c in range(NCHUNKS):
                        nc.tensor.matmul(
                            ps[m][:, c * CHUNK:(c + 1) * CHUNK],
                            lhsT=weights[j][:, m * P:(m + 1) * P],
                            rhs=rhs_tiles[j][:, c * CHUNK:(c + 1) * CHUNK],
                            start=(start and j == 0),
                            stop=(stop and j == nkt - 1),
                            skip_group_check=True,
                        )

            # HAM warm-up: junk matmuls on a memset tile keep the PE busy
            # while inputs stream in, so the real matmuls run at 2.4 GHz
            # from the start (the activity monitor needs ~3.4us of sustained
            # work to unthrottle). Results are overwritten by the first
            # start=True matmul per bank.
            junk = wpool.tile([P, N], bf16, tag="junk", name="junk")
            nc.gpsimd.memset(junk[:], 0)
            for r in range(20):
                nc.tensor.matmul(
                    ps[r % MT][:, 0:CHUNK],
                    lhsT=junk[:, 0:P], rhs=junk[:, 0:CHUNK],
                    start=True, stop=True, skip_group_check=True,
                )

            # init: psum = x @ W_top + s0 @ W_bot
            mm_round(wt, x_sb, start=True, stop=False)
            mm_round(wbot, s_sb, start=False, stop=False)

            # ---- unfolds ---------------------------------------------------
            for k in range(UNFOLDS):
                last = k == UNFOLDS - 1
                tmp_t = [tmpp.tile([P, BC], bf16, tag=f"tmp{j}",
                                   name=f"tmp{k}_{j}")
                         for j in range(MT)]
                f_t = [fpool.tile([P, BC], f32, tag=f"f{m}", name=f"f{k}_{m}",
                                  bufs=2)
                       for m in range(MT)]
                for m in range(MT):
                    # f = tanh(psum + bias), full (128,1024) span
                    nc.scalar.activation(
                        f_t[m][:], ps[m][:], TANH,
                        bias=bias_sb[:, m:m + 1], scale=1.0,
                    )
                    # tmp = f - s (bf16 out, feeds the delta matmuls)
                    nc.vector.scalar_tensor_tensor(
                        tmp_t[m][:], s_sb[m][:], -1.0, f_t[m][:],
                        op0=MULT, op1=ADD,
                    )
                    if last:
                        # final state + output DMA per m-tile, ASAP
                        nc.vector.scalar_tensor_tensor(
                            s_sb[m][:], tmp_t[m][:], DT, s_sb[m][:],
                            op0=MULT, op1=ADD,
                        )
                        out_eng = (nc.sync, nc.scalar, nc.gpsimd,
                                   nc.sync)[m]
                        out_eng.dma_start(outT[m * P:(m + 1) * P, :],
                                          s_sb[m][:].bitcast(f32))
                if not last:
                    # psum += tmp @ (0.1*W_bot)   [bf16]
                    mm_round(wb01h, tmp_t, start=False,
                             stop=(k == UNFOLDS - 2))
                    # s += 0.1 * tmp  (emitted after the matmuls: off the
                    # critical path, fills DVE gaps)
                    for m in range(MT):
                        nc.vector.scalar_tensor_tensor(
                            s_sb[m][:], tmp_t[m][:], DT, s_sb[m][:],
                            op0=MULT, op1=ADD,
                        )

    nc.compile()
    return nc


def _get_nc():
    global _compiled_nc
    if _compiled_nc is None:
        _compiled_nc = _build_nc()
    return _compiled_nc


def make_in_maps(x, s, W, b):
    """Shard + pack host-side: transposed x/s, W packed to (128, 4096) with
    4KB-contiguous per-partition runs (k-tile pairs side by side)."""
    xT = np.ascontiguousarray(x.T)   # (D, B)
    sTf = np.ascontiguousarray(s.T)  # (N, B)
    Wp = np.ascontiguousarray(
        W.reshape(4, 2, P, N).transpose(2, 0, 1, 3).reshape(P, -1))
    in_maps = []
    for c in range(NCORES):
        sl = slice(c * BC, (c + 1) * BC)
        in_maps.append({
            "xT": np.ascontiguousarray(xT[:, sl]),
            "sT": np.ascontiguousarray(sTf[:, sl]),
            "Wp": Wp,
            "bias": b,
        })
    return in_maps


def kernel(**inputs):
    from concourse.bass_utils import run_bass_kernel_spmd

    x = np.asarray(inputs["inputs"], dtype=np.float32)
    s = np.asarray(inputs["state"], dtype=np.float32)
    W = np.ascontiguousarray(np.asarray(inputs["W"], dtype=np.float32))
    b = np.ascontiguousarray(np.asarray(inputs["bias"], dtype=np.float32))

    in_maps = make_in_maps(x, s, W, b)
    nc = _get_nc()
    res = run_bass_kernel_spmd(nc, in_maps, list(range(NCORES))).results
    outT = np.concatenate([res[c]["outT"] for c in range(NCORES)], axis=1)
    out = np.ascontiguousarray(outT.T).astype(np.float32)
    return (out, out)
